# revision 31
# baseline (speedup 1.0000x reference)
"""AFD channel attention on 8 TRN2 NeuronCores.

Math (per row r of x_flat [B*C, L], L = 64*64 = 4096, N = 64 basis fns):
    proj = x_flat @ B.T            [BC, N]
    w    = softmax(|proj|, -1)     [BC, N]
    out  = x_flat + w @ B          [BC, L]

Strategy: data-parallel over the 16384 (b, c) rows, 2048 rows per core.
Everything on-device runs in the TRANSPOSED domain (outT = xT + attnT) so
that the contraction dim of both matmuls lies on SBUF partitions with no
on-chip transpose of the big tensor.

v3 design (baseline was bf16-in/bf16-out at the 33.6MB DMA roofline,
~112us fast-window / ~152us down-throttled):
  - uint8 OUTPUT wire format: the device computes q = (x + attn - c)/s
    and ships q; the host decodes out = q*s + c. The affine offset folds
    into the attn basis (B_attn = (B - c)/s works because the softmax
    weights sum to 1) and the scale into the input (x' = x/s with
    B_proj = s*B keeps proj bit-identical), so the epilogue stays one
    tensor_tensor add per tile. f32->uint8 conversion on DVE rounds to
    nearest-even and saturates (probe-verified). Output bytes halve:
    25.9MB -> ~72us DMA floor.
  - fp16 (e5m10) instead of bf16 for x' and the bases: same bytes, 8x
    less input rounding -> far fewer softmax argmax flips (the error is
    dominated by winner-take-all flips), buying margin for the uint8
    quantization. Total rel err ~1.4e-2 vs the 2e-2 budget.
  - attn matmuls fill f32 PSUM [128, 1024] tiles (2 banks, 2 bank-wide
    matmuls each); the epilogue is slab-typed: "A" slabs are single wide
    DVE ops (psum + x' -> uint8 direct, HWDGE out DMA), "B" slabs use
    ACT psum->fp16 copies with GPSIMD/DVE adds into fp16 tiles that the
    SWDGE out-DMA casts to uint8 in flight (Pool cannot write uint8).
    Slice 1 (the exposed tail) leans on "B" so DVE/ACT/GPSIMD all carry
    it when the HAM down-throttle (k=4 after the ~50-65us full-rate
    window) halves every engine.
  - DRAM layouts packed [NS, NJ, 128, J, SBC] so every DMA is one
    [128, J*SBC] slab with 4-8KB contiguous per-partition descriptors
    (>= the 4KB bus-saturation knee); 16 input + 16 output DMAs.
  - slice-0's attn+epilogue INTERLEAVES with slice-1's input-gated proj
    on the PE stream — each attn(s0) slab is emitted BEFORE the proj(s1)
    chunks it pairs with, so the in-order PE never stalls attn(s0)
    behind s1's input arrivals and half the epilogue hides under the
    input stream; out tiles share the x'-slab pool ring (slot of the
    slab consumed two attn-slabs earlier) to fit SBUF.
  - HAM warm-up matmuls bridge the input-latency bubble so real matmuls
    start at 2.4 GHz.
"""

import sys

for p in ("/opt/trn_rl_repo", "/root/.axon_site/_ro/trn_rl_repo"):
    if p not in sys.path:
        sys.path.append(p)

import numpy as np

import concourse.bass as bass
import concourse.mybir as mybir
import concourse.tile as tile
from concourse.bass_utils import run_bass_kernel_spmd

F16 = mybir.dt.float16
F32 = mybir.dt.float32
U8 = mybir.dt.uint8

N_BASIS = 64
R = 0.9
L = 4096            # 64 * 64
BC_TOTAL = 16384    # 32 * 512
N_CORES = 8
BC = BC_TOTAL // N_CORES   # 2048 rows per core
KC = L // 128       # 32 l-chunks of 128

NS = 2              # BC slices per core (phase pipelining)
SBC = BC // NS      # 1024 columns per slice
SNG = SBC // 512    # 2 proj psum groups of 512 per slice
SNT = SBC // 128    # 8 softmax tiles per slice
J = 4               # l-chunks per DMA slab
NJ = KC // J        # 8 slabs per slice

# Epilogue slab types (one slab = J units of [128, SBC]):
#   "A": DVE adds psum+x' -> uint8 directly (1x), HWDGE out DMA.
#   "B": ACT copies psum->fp16; adds on GPSIMD or DVE(2x) -> fp16 tile;
#        SWDGE cast-DMA converts fp16->uint8 in flight (Pool cannot
#        write uint8, and DVE's u8-write runs at 1x; the split keeps all
#        three engines fed).
# per (slice, slab): slice 0's epilogue hides under the input stream, so
# it stays lean all-DVE; slice 1 is the exposed tail, so it spreads
# across DVE/ACT/GPSIMD to run at parallel-engine pace when throttled.
SLAB_TYPES = (
    ("A", "A", "B", "A", "A", "A", "B", "A"),
    ("A", "B", "B", "A", "B", "B", "A", "B"),
)
B_UNIT = ("gps", "dve2", "gps", "dve2")                 # per unit in B slab


def _blaschke_basis_f64(length):
    thetas = np.linspace(0.0, 2.0 * np.pi, N_BASIS, endpoint=False)
    t = np.linspace(0.0, 2.0 * np.pi, length)
    cosp = np.cos(t[None, :] - thetas[:, None])
    scale = np.sqrt(1.0 - R * R)
    return scale * (1.0 - R * cosp) / (1.0 - 2.0 * R * cosp + R * R)  # [N, L]


def _build():
    nc = bass.Bass()
    # x' = x/s in fp16, packed [NS, NJ, 128, J, SBC]:
    #   value at [s, nj, p, j, c] = xT[128*(nj*J + j) + p, s*SBC + c]
    xt_ext = nc.declare_dram_parameter(
        "xt", [NS, NJ, 128, J, SBC], F16, isOutput=False
    )
    # bt host-packed [128, KC*64]: bt[p, 64k+n] = s*B[n, 128k+p], so one
    # contiguous DMA loads every proj lhsT chunk.
    bt_ext = nc.declare_dram_parameter("bt", [128, KC * N_BASIS], F16, isOutput=False)
    # bn = (B - c)/s  [N, L]
    bn_ext = nc.declare_dram_parameter("bn", [N_BASIS, L], F16, isOutput=False)
    id_ext = nc.declare_dram_parameter("ident", [128, 128], F32, isOutput=False)
    # q = (x + attn - c)/s as uint8, same [NS, NJ, 128, J, SBC] packing
    out_ext = nc.declare_dram_parameter(
        "out", [NS, NJ, 128, J, SBC], U8, isOutput=True
    )

    add = mybir.AluOpType.add
    X = mybir.AxisListType.X
    Act = mybir.ActivationFunctionType

    with tile.TileContext(nc) as tc:
        with (
            # 16 input slabs + 2 spare slots: out tiles allocate from the
            # same tag ring, so out slab (s, nj) recycles the slot of the
            # x' slab consumed two attn-slabs earlier (program order on
            # every engine makes the WAR safe; Tile adds the sems).
            tc.tile_pool(name="xt", bufs=NS * NJ + 2) as xt_pool,
            tc.tile_pool(name="const", bufs=1) as cpool,
            tc.tile_pool(name="smproj", bufs=2) as smproj_pool,
            tc.tile_pool(name="sm", bufs=6) as sm_pool,
            tc.tile_pool(name="attnsb", bufs=4) as attnsb_pool,
            tc.tile_pool(name="ps_proj", bufs=SNG, space="PSUM") as ps_proj,
            tc.tile_pool(name="ps_t", bufs=1, space="PSUM") as ps_t,
            tc.tile_pool(name="ps_at", bufs=5, space="PSUM") as ps_at,
        ):
            # -- constants (each a single contiguous DMA) --
            bt_sb = cpool.tile([128, KC * N_BASIS], F16)   # chunk k at [:, 64k:64k+64]
            bn_sb = cpool.tile([N_BASIS, L], F16)
            id_sb = cpool.tile([128, 128], F32)
            idh_sb = cpool.tile([128, 128], F16)
            nc.sync.dma_start(bt_sb[:], bt_ext[:])
            nc.sync.dma_start(bn_sb[:], bn_ext[:])
            nc.sync.dma_start(id_sb[:], id_ext[:])
            nc.vector.tensor_copy(idh_sb[:], id_sb[:])

            # -- HAM warm-up: dummy accumulating matmuls on the basis tile
            # while the first input slabs are in flight, so the PE clock is
            # at 8/8 when real work arrives.
            warm_ps = ps_at.tile([128, 512], F32, tag="at", name="warm")
            N_WARM = 44
            for i in range(N_WARM):
                nc.tensor.matmul(
                    warm_ps[:N_BASIS, :],
                    bt_sb[:, :N_BASIS],
                    bt_sb[:, 512:1024],
                    start=(i == 0), stop=(i == N_WARM - 1),
                )

            # -- all input DMAs issued up-front; one DMA per (s, nj) slab --
            xt_tiles = {}
            for s in range(NS):
                for nj in range(NJ):
                    xt_t = xt_pool.tile(
                        [128, J * SBC], F16, tag="xt", name=f"xt{s}_{nj}"
                    )
                    nc.sync.dma_start(
                        xt_t[:].rearrange("p (j c) -> p j c", j=J),
                        xt_ext[s, nj],
                    )
                    xt_tiles[(s, nj)] = xt_t

            def xchunk(s, k, gs):
                off = (k % J) * SBC
                return xt_tiles[(s, k // J)][:, off + gs.start:off + gs.stop]

            proj_ps_s = {}

            def emit_proj_chunk(s, k):
                if k == 0:
                    proj_ps_s[s] = [
                        ps_proj.tile(
                            [N_BASIS, 512], F32, tag="proj", name=f"proj{s}_{g}"
                        )
                        for g in range(SNG)
                    ]
                proj_ps = proj_ps_s[s]
                for g in range(SNG):
                    gs = slice(g * 512, (g + 1) * 512)
                    nc.tensor.matmul(
                        proj_ps[g][:],
                        bt_sb[:, k * N_BASIS:(k + 1) * N_BASIS],
                        xchunk(s, k, gs),
                        start=(k == 0), stop=(k == KC - 1),
                    )

            def emit_softmax(s):
                proj_ps = proj_ps_s[s]
                projT_sb = smproj_pool.tile(
                    [N_BASIS, SBC], F32, tag="projT", name=f"projT{s}"
                )
                wT_sb = smproj_pool.tile(
                    [N_BASIS, SBC], F16, tag="wT", name=f"wT{s}"
                )
                for g in range(SNG):
                    gs = slice(g * 512, (g + 1) * 512)
                    nc.scalar.copy(projT_sb[:, gs], proj_ps[g][:])
                for t in range(SNT):
                    ts = slice(t * 128, (t + 1) * 128)
                    pt = ps_t.tile([128, N_BASIS], F32, tag="pt", name="pt")
                    nc.tensor.transpose(
                        pt[:], projT_sb[:, ts], id_sb[:N_BASIS, :N_BASIS]
                    )
                    negmax = sm_pool.tile([128, 1], F32, tag="negmax")
                    nc.vector.reduce_max(
                        negmax[:], pt[:], axis=X,
                        apply_absolute_value=True, negate=True,
                    )
                    absp = sm_pool.tile([128, N_BASIS], F32, tag="absp")
                    nc.scalar.activation(absp[:], pt[:], Act.Abs)
                    expv = sm_pool.tile([128, N_BASIS], F32, tag="expv")
                    sumexp = sm_pool.tile([128, 1], F32, tag="sumexp")
                    nc.scalar.activation(
                        expv[:], absp[:], Act.Exp, bias=negmax[:], scale=1.0,
                        accum_out=sumexp[:],
                    )
                    rsum = sm_pool.tile([128, 1], F32, tag="rsum")
                    nc.vector.reciprocal(rsum[:], sumexp[:])
                    wfb = sm_pool.tile([128, N_BASIS], F16, tag="wfb")
                    nc.vector.tensor_scalar_mul(wfb[:], expv[:], rsum[:])
                    wt_ps = ps_t.tile([N_BASIS, 128], F16, tag="pt", name="wt")
                    nc.tensor.transpose(wt_ps[:], wfb[:], idh_sb[:])
                    nc.vector.tensor_copy(wT_sb[:, ts], wt_ps[:])
                return wT_sb

            def emit_attn_slab(s, nj, wT_sb):
                # one f32 psum tile [128, SBC] (2 banks) per l-chunk, filled
                # by 2 bank-sized matmuls; epilogue is one wide op per chunk
                # (f32->uint8 / ->fp16 conversion rounds to nearest-even and
                # saturates).
                stype = SLAB_TYPES[s][nj]
                out_t = xt_pool.tile(
                    [128, J * SBC], U8 if stype == "A" else F16,
                    tag="xt", name=f"out{s}_{nj}",
                )
                for j in range(J):
                    k = nj * J + j
                    for g in range(SNG):
                        gs = slice(g * 512, (g + 1) * 512)
                        # one 1-bank psum tile per unit, 5-deep ring: keeps
                        # several matmul->epilogue chains in flight so the
                        # engines pipeline instead of ping-ponging
                        at_ps = ps_at.tile([128, 512], F32, tag="at")
                        nc.tensor.matmul(
                            at_ps[:],
                            bn_sb[:, k * 128:(k + 1) * 128],
                            wT_sb[:, gs],
                            start=True, stop=True,
                        )
                        lo = j * SBC
                        o_sl = out_t[:, lo + gs.start:lo + gs.stop]
                        x_sl = xt_tiles[(s, nj)][:, lo + gs.start:lo + gs.stop]
                        if stype == "A":
                            nc.vector.tensor_tensor(o_sl, at_ps[:], x_sl, op=add)
                        else:
                            attn_sb = attnsb_pool.tile(
                                [128, 512], F16, tag="attnsb"
                            )
                            nc.scalar.copy(attn_sb[:], at_ps[:])
                            path = B_UNIT[(j * SNG + g) % len(B_UNIT)]
                            eng = nc.vector if path == "dve2" else nc.gpsimd
                            eng.tensor_tensor(o_sl, attn_sb[:], x_sl, op=add)
                # A slabs ship uint8 straight (HWDGE); B slabs cast
                # fp16->uint8 inside the DMA (SWDGE)
                dma_eng = nc.sync if stype == "A" else nc.gpsimd
                dma_eng.dma_start(
                    out_ext[s, nj],
                    out_t[:].rearrange("p (j c) -> p j c", j=J),
                )

            # Phase order: proj(s0); softmax(s0); then proj(s1) (input-
            # gated) INTERLEAVED with attn(s0) on the PE stream so the
            # epilogue of slice 0 hides under the input stream; softmax(s1);
            # attn(s1) dense.
            for k in range(KC):
                emit_proj_chunk(0, k)
            wt0 = emit_softmax(0)
            # attn(s0) slab BEFORE the proj(s1) chunks it pairs with: its
            # deps (input s0 + wt0) are already met, so the in-order PE
            # stream never stalls attn(s0) behind s1's input arrivals.
            for nj in range(NJ):
                emit_attn_slab(0, nj, wt0)
                for j in range(J):
                    emit_proj_chunk(1, nj * J + j)
            wt1 = emit_softmax(1)
            for nj in range(NJ):
                emit_attn_slab(1, nj, wt1)

    return nc


def _split_multi_waits(bir: bytes) -> bytes:
    """This walrus build caps sync waits at ONE per instruction
    (CoreV3GenImpl setupSyncWait: 'Too many sync wait commands'), but Tile
    emits multi-sem waits. Hoist the extras onto wait-only EventSemaphore
    carriers placed just before the instruction on the same engine —
    program order makes the split semantically identical to the fused
    multi-wait."""
    import orjson

    m = orjson.loads(bir)
    n = 0
    for f in m["functions"]:
        for blk in f["blocks"]:
            insts = blk.get("instructions")
            if not insts:
                continue
            out = []
            changed = False
            for ins in insts:
                si = ins.get("sync_info")
                ow = (si or {}).get("on_wait") or []
                if len(ow) > 1:
                    changed = True
                    for w in ow[:-1]:
                        n += 1
                        out.append(
                            {
                                "debug": ins.get("debug"),
                                "engine": ins["engine"],
                                "ins": [],
                                "outs": [],
                                "name": f"waitsplit-{n}",
                                "opcode": "EventSemaphore",
                                "sync_info": {"on_update": [], "on_wait": [w]},
                            }
                        )
                    si["on_wait"] = [ow[-1]]
                out.append(ins)
            if changed:
                blk["instructions"] = out
    return orjson.dumps(m)


_NC_CACHE = {}


def _get_nc():
    if "nc" not in _NC_CACHE:
        nc = _build()
        orig_to_json = nc.to_json_bytes
        nc.to_json_bytes = lambda: _split_multi_waits(orig_to_json())
        _NC_CACHE["nc"] = nc
    return _NC_CACHE["nc"]


def kernel(x, _trace=False, _tmpdir=None):
    assert x.shape == (32, 512, 64, 64) and x.dtype == np.float32
    x_flat = np.ascontiguousarray(x.reshape(BC_TOTAL, L))

    B64 = _blaschke_basis_f64(L)
    b_lo, b_hi = B64.min(), B64.max()
    ident = np.eye(128, dtype=np.float32)

    in_maps = []
    scales = []
    for i in range(N_CORES):
        shard = x_flat[i * BC:(i + 1) * BC]                 # [BC, L] f32
        x_lo, x_hi = float(shard.min()), float(shard.max())
        lo, hi = x_lo + b_lo, x_hi + b_hi                   # bounds on out
        s = (hi - lo) / 250.0
        c = lo - 2.5 * s                                    # q spans [2.5, 252.5]
        scales.append((s, c))

        # x' = x/s packed [NS, NJ, 128, J, SBC]
        xt = np.ascontiguousarray(shard.T) * np.float32(1.0 / s)   # [L, BC]
        xt = xt.reshape(NJ, J, 128, NS, SBC).transpose(3, 0, 2, 1, 4)
        xt = np.ascontiguousarray(xt).astype(np.float16)

        bn = np.ascontiguousarray((B64 - c) / s).astype(np.float16)  # [N, L]
        # packed bt: [128, KC*64] with bt[p, 64k+n] = s*B[n, 128k+p]
        bt = np.ascontiguousarray(
            (s * B64).T.reshape(KC, 128, N_BASIS)
            .transpose(1, 0, 2).reshape(128, KC * N_BASIS)
        ).astype(np.float16)
        in_maps.append({"xt": xt, "bt": bt, "bn": bn, "ident": ident})

    nc = _get_nc()
    res = run_bass_kernel_spmd(
        nc, in_maps, core_ids=list(range(N_CORES)), trace=_trace, tmpdir=_tmpdir
    )

    outs = []
    for i in range(N_CORES):
        s, c = scales[i]
        q = np.asarray(res.results[i]["out"])       # [NS, NJ, 128, J, SBC] u8
        # l = 128*(nj*J + j) + p; col = s_idx*SBC + c_idx
        qt = q.transpose(1, 3, 2, 0, 4).reshape(L, BC)      # [L, BC]
        out = qt.T.astype(np.float32) * np.float32(s) + np.float32(c)
        outs.append(out)                                    # [BC, L]
    out = np.concatenate(outs, axis=0).reshape(32, 512, 64, 64)
    if _trace:
        return out, res
    return out


# revision 51
# speedup vs baseline: 1.1174x; 1.1174x over previous
"""AFD channel attention on 8 TRN2 NeuronCores.

Math (per row r of x_flat [B*C, L], L = 64*64 = 4096, N = 64 basis fns):
    proj = x_flat @ B.T            [BC, N]
    w    = softmax(|proj|, -1)     [BC, N]
    out  = x_flat + w @ B          [BC, L]

Strategy: data-parallel over the 16384 (b, c) rows, 2048 rows per core.
Everything on-device runs in the TRANSPOSED domain (outT = xT + attnT) so
that the contraction dim of both matmuls lies on SBUF partitions with no
on-chip transpose of the big tensor.

v3 design (baseline was bf16-in/bf16-out at the 33.6MB DMA roofline,
~112us fast-window / ~152us down-throttled):
  - uint8 OUTPUT wire format: the device computes q = (x + attn - c)/s
    and ships q; the host decodes out = q*s + c. The affine offset folds
    into the attn basis (B_attn = (B - c)/s works because the softmax
    weights sum to 1) and the scale into the input (x' = x/s with
    B_proj = s*B keeps proj bit-identical), so the epilogue stays one
    tensor_tensor add per tile. f32->uint8 conversion on DVE rounds to
    nearest-even and saturates (probe-verified). Output bytes halve:
    25.9MB -> ~72us DMA floor.
  - fp16 (e5m10) instead of bf16 for x' and the bases: same bytes, 8x
    less input rounding -> far fewer softmax argmax flips (the error is
    dominated by winner-take-all flips), buying margin for the uint8
    quantization. Total rel err ~1.4e-2 vs the 2e-2 budget.
  - attn matmuls fill f32 PSUM [128, 1024] tiles (2 banks, 2 bank-wide
    matmuls each); the epilogue is slab-typed: "A" slabs are single wide
    DVE ops (psum + x' -> uint8 direct, HWDGE out DMA), "B" slabs use
    ACT psum->fp16 copies with GPSIMD/DVE adds into fp16 tiles that the
    SWDGE out-DMA casts to uint8 in flight (Pool cannot write uint8).
    Slice 1 (the exposed tail) leans on "B" so DVE/ACT/GPSIMD all carry
    it when the HAM down-throttle (k=4 after the ~50-65us full-rate
    window) halves every engine.
  - DRAM layouts packed [NS, NJ, 128, J, SBC] so every DMA is one
    [128, J*SBC] slab with 4-8KB contiguous per-partition descriptors
    (>= the 4KB bus-saturation knee); 16 input + 16 output DMAs.
  - a QUARTER of slice-0's attn+epilogue interleaves with slice-1's
    input-gated proj on the PE stream (each attn(s0) slab emitted BEFORE
    the proj(s1) chunks it pairs with, so the in-order PE never stalls
    attn(s0) behind s1's input arrivals); the rest defers to after
    softmax(s1) — the input window only has ~22us of PE headroom and
    overloading it pushes softmax(s1) past the HAM window's end. The
    exposed tail ends on DVE-paced "A" slabs (GPSIMD is the slowest
    engine when throttled). Out tiles share the x'-slab pool ring (slot
    of the slab consumed two attn-slabs earlier) to fit SBUF.
  - HAM warm-up matmuls bridge the input-latency bubble so real matmuls
    start at 2.4 GHz.
"""

import sys

for p in ("/opt/trn_rl_repo", "/root/.axon_site/_ro/trn_rl_repo"):
    if p not in sys.path:
        sys.path.append(p)

import numpy as np

import concourse.bass as bass
import concourse.mybir as mybir
import concourse.tile as tile
from concourse.bass_utils import run_bass_kernel_spmd

F16 = mybir.dt.float16
F32 = mybir.dt.float32
U8 = mybir.dt.uint8

N_BASIS = 64
R = 0.9
L = 4096            # 64 * 64
BC_TOTAL = 16384    # 32 * 512
N_CORES = 8
BC = BC_TOTAL // N_CORES   # 2048 rows per core
KC = L // 128       # 32 l-chunks of 128

NS = 2              # BC slices per core (phase pipelining)
SBC = BC // NS      # 1024 columns per slice
SNG = SBC // 512    # 2 proj psum groups of 512 per slice
SNT = SBC // 128    # 8 softmax tiles per slice
J = 4               # l-chunks per DMA slab
NJ = KC // J        # 8 slabs per slice

# Epilogue slab types (one slab = J units of [128, SBC]):
#   "A": DVE adds psum+x' -> uint8 directly (1x), HWDGE out DMA.
#   "B": ACT copies psum->fp16; adds on GPSIMD or DVE(2x) -> fp16 tile;
#        SWDGE cast-DMA converts fp16->uint8 in flight (Pool cannot
#        write uint8, and DVE's u8-write runs at 1x; the split keeps all
#        three engines fed).
# per (slice, slab): slice 0's epilogue hides under the input stream, so
# it stays lean all-DVE; slice 1 is the exposed tail, so it spreads
# across DVE/ACT/GPSIMD to run at parallel-engine pace when throttled.
SLAB_TYPES = (
    ("A", "A", "B", "A", "A", "A", "B", "A"),
    ("B", "B", "A", "B", "B", "A", "A", "A"),
)
B_UNIT = ("gps", "dve2", "gps", "dve2")                 # per unit in B slab


def _blaschke_basis_f64(length):
    thetas = np.linspace(0.0, 2.0 * np.pi, N_BASIS, endpoint=False)
    t = np.linspace(0.0, 2.0 * np.pi, length)
    cosp = np.cos(t[None, :] - thetas[:, None])
    scale = np.sqrt(1.0 - R * R)
    return scale * (1.0 - R * cosp) / (1.0 - 2.0 * R * cosp + R * R)  # [N, L]


def _build():
    nc = bass.Bass()
    # x' = x/s in fp16, packed [NS, NJ, 128, J, SBC]:
    #   value at [s, nj, p, j, c] = xT[128*(nj*J + j) + p, s*SBC + c]
    xt_ext = nc.declare_dram_parameter(
        "xt", [NS, NJ, 128, J, SBC], F16, isOutput=False
    )
    # bt host-packed [128, KC*64]: bt[p, 64k+n] = s*B[n, 128k+p], so one
    # contiguous DMA loads every proj lhsT chunk.
    bt_ext = nc.declare_dram_parameter("bt", [128, KC * N_BASIS], F16, isOutput=False)
    # bn = (B - c)/s  [N, L]
    bn_ext = nc.declare_dram_parameter("bn", [N_BASIS, L], F16, isOutput=False)
    id_ext = nc.declare_dram_parameter("ident", [128, 128], F32, isOutput=False)
    # q = (x + attn - c)/s as uint8, same [NS, NJ, 128, J, SBC] packing
    out_ext = nc.declare_dram_parameter(
        "out", [NS, NJ, 128, J, SBC], U8, isOutput=True
    )

    add = mybir.AluOpType.add
    X = mybir.AxisListType.X
    Act = mybir.ActivationFunctionType

    with tile.TileContext(nc) as tc:
        with (
            # 16 input slabs + 2 spare slots: out tiles allocate from the
            # same tag ring, so out slab (s, nj) recycles the slot of the
            # x' slab consumed two attn-slabs earlier (program order on
            # every engine makes the WAR safe; Tile adds the sems).
            tc.tile_pool(name="xt", bufs=NS * NJ + 2) as xt_pool,
            tc.tile_pool(name="const", bufs=1) as cpool,
            tc.tile_pool(name="smproj", bufs=2) as smproj_pool,
            tc.tile_pool(name="sm", bufs=6) as sm_pool,
            tc.tile_pool(name="attnsb", bufs=4) as attnsb_pool,
            tc.tile_pool(name="ps_proj", bufs=SNG, space="PSUM") as ps_proj,
            tc.tile_pool(name="ps_t", bufs=1, space="PSUM") as ps_t,
            tc.tile_pool(name="ps_at", bufs=2, space="PSUM") as ps_at,
        ):
            # -- constants (each a single contiguous DMA) --
            bt_sb = cpool.tile([128, KC * N_BASIS], F16)   # chunk k at [:, 64k:64k+64]
            bn_sb = cpool.tile([N_BASIS, L], F16)
            id_sb = cpool.tile([128, 128], F32)
            idh_sb = cpool.tile([128, 128], F16)
            nc.sync.dma_start(bt_sb[:], bt_ext[:])
            nc.sync.dma_start(bn_sb[:], bn_ext[:])
            nc.sync.dma_start(id_sb[:], id_ext[:])
            nc.vector.tensor_copy(idh_sb[:], id_sb[:])

            # -- HAM warm-up: dummy accumulating matmuls on the basis tile
            # while the first input slabs are in flight, so the PE clock is
            # at 8/8 when real work arrives.
            warm_ps = ps_at.tile([128, SBC], F32, tag="at", name="warm")
            N_WARM = 24
            for i in range(N_WARM):
                nc.tensor.matmul(
                    warm_ps[:N_BASIS, :512],
                    bt_sb[:, :N_BASIS],
                    bt_sb[:, 512:1024],
                    start=(i == 0), stop=(i == N_WARM - 1),
                )

            # -- all input DMAs issued up-front; one DMA per (s, nj) slab --
            xt_tiles = {}
            for s in range(NS):
                for nj in range(NJ):
                    xt_t = xt_pool.tile(
                        [128, J * SBC], F16, tag="xt", name=f"xt{s}_{nj}"
                    )
                    nc.sync.dma_start(
                        xt_t[:].rearrange("p (j c) -> p j c", j=J),
                        xt_ext[s, nj],
                    )
                    xt_tiles[(s, nj)] = xt_t

            def xchunk(s, k, gs):
                off = (k % J) * SBC
                return xt_tiles[(s, k // J)][:, off + gs.start:off + gs.stop]

            proj_ps_s = {}

            def emit_proj_chunk(s, k):
                if k == 0:
                    proj_ps_s[s] = [
                        ps_proj.tile(
                            [N_BASIS, 512], F32, tag="proj", name=f"proj{s}_{g}"
                        )
                        for g in range(SNG)
                    ]
                proj_ps = proj_ps_s[s]
                for g in range(SNG):
                    gs = slice(g * 512, (g + 1) * 512)
                    nc.tensor.matmul(
                        proj_ps[g][:],
                        bt_sb[:, k * N_BASIS:(k + 1) * N_BASIS],
                        xchunk(s, k, gs),
                        start=(k == 0), stop=(k == KC - 1),
                    )

            def emit_softmax(s):
                proj_ps = proj_ps_s[s]
                projT_sb = smproj_pool.tile(
                    [N_BASIS, SBC], F32, tag="projT", name=f"projT{s}"
                )
                wT_sb = smproj_pool.tile(
                    [N_BASIS, SBC], F16, tag="wT", name=f"wT{s}"
                )
                for g in range(SNG):
                    gs = slice(g * 512, (g + 1) * 512)
                    nc.scalar.copy(projT_sb[:, gs], proj_ps[g][:])
                for t in range(SNT):
                    ts = slice(t * 128, (t + 1) * 128)
                    pt = ps_t.tile([128, N_BASIS], F32, tag="pt", name="pt")
                    nc.tensor.transpose(
                        pt[:], projT_sb[:, ts], id_sb[:N_BASIS, :N_BASIS]
                    )
                    negmax = sm_pool.tile([128, 1], F32, tag="negmax")
                    nc.vector.reduce_max(
                        negmax[:], pt[:], axis=X,
                        apply_absolute_value=True, negate=True,
                    )
                    absp = sm_pool.tile([128, N_BASIS], F32, tag="absp")
                    nc.scalar.activation(absp[:], pt[:], Act.Abs)
                    expv = sm_pool.tile([128, N_BASIS], F32, tag="expv")
                    sumexp = sm_pool.tile([128, 1], F32, tag="sumexp")
                    nc.scalar.activation(
                        expv[:], absp[:], Act.Exp, bias=negmax[:], scale=1.0,
                        accum_out=sumexp[:],
                    )
                    rsum = sm_pool.tile([128, 1], F32, tag="rsum")
                    nc.vector.reciprocal(rsum[:], sumexp[:])
                    wfb = sm_pool.tile([128, N_BASIS], F16, tag="wfb")
                    nc.vector.tensor_scalar_mul(wfb[:], expv[:], rsum[:])
                    wt_ps = ps_t.tile([N_BASIS, 128], F16, tag="wt", name="wt")
                    nc.tensor.transpose(wt_ps[:], wfb[:], idh_sb[:])
                    nc.vector.tensor_copy(wT_sb[:, ts], wt_ps[:])
                return wT_sb

            def emit_attn_slab(s, nj, wT_sb):
                # one f32 psum tile [128, SBC] (2 banks) per l-chunk, filled
                # by 2 bank-sized matmuls; epilogue is one wide op per chunk
                # (f32->uint8 / ->fp16 conversion rounds to nearest-even and
                # saturates).
                stype = SLAB_TYPES[s][nj]
                out_t = xt_pool.tile(
                    [128, J * SBC], U8 if stype == "A" else F16,
                    tag="xt", name=f"out{s}_{nj}",
                )
                for j in range(J):
                    k = nj * J + j
                    at_ps = ps_at.tile([128, SBC], F32, tag="at")
                    for g in range(SNG):
                        gs = slice(g * 512, (g + 1) * 512)
                        nc.tensor.matmul(
                            at_ps[:, gs],
                            bn_sb[:, k * 128:(k + 1) * 128],
                            wT_sb[:, gs],
                            start=True, stop=True,
                        )
                    lo = j * SBC
                    o_sl = out_t[:, lo:lo + SBC]
                    x_sl = xt_tiles[(s, nj)][:, lo:lo + SBC]
                    if stype == "A":
                        nc.vector.tensor_tensor(o_sl, at_ps[:], x_sl, op=add)
                    else:
                        attn_sb = attnsb_pool.tile([128, SBC], F16, tag="attnsb")
                        nc.scalar.copy(attn_sb[:], at_ps[:])
                        eng = nc.vector if B_UNIT[j] == "dve2" else nc.gpsimd
                        eng.tensor_tensor(o_sl, attn_sb[:], x_sl, op=add)
                # A slabs ship uint8 straight (HWDGE); B slabs cast
                # fp16->uint8 inside the DMA (SWDGE)
                dma_eng = nc.sync if stype == "A" else nc.gpsimd
                dma_eng.dma_start(
                    out_ext[s, nj],
                    out_t[:].rearrange("p (j c) -> p j c", j=J),
                )

            # Phase order: proj(s0); softmax(s0); then proj(s1) (input-
            # gated) INTERLEAVED with attn(s0) on the PE stream so the
            # epilogue of slice 0 hides under the input stream; softmax(s1);
            # attn(s1) dense.
            for k in range(KC):
                emit_proj_chunk(0, k)
            wt0 = emit_softmax(0)
            # attn(s0) slab BEFORE the proj(s1) chunks it pairs with: its
            # deps (input s0 + wt0) are already met, so the in-order PE
            # stream never stalls attn(s0) behind s1's input arrivals.
            # only half of attn(s0) interleaves with proj(s1): the input
            # window has ~22us of PE headroom, and overloading it delays
            # softmax(s1) past the HAM full-rate window's end
            for nj in range(NJ):
                if nj < NJ // 4:
                    emit_attn_slab(0, nj, wt0)
                for j in range(J):
                    emit_proj_chunk(1, nj * J + j)
            wt1 = emit_softmax(1)
            for nj in range(NJ // 4, NJ):
                emit_attn_slab(0, nj, wt0)
            for nj in range(NJ):
                emit_attn_slab(1, nj, wt1)

    return nc


def _split_multi_waits(bir: bytes) -> bytes:
    """This walrus build caps sync waits at ONE per instruction
    (CoreV3GenImpl setupSyncWait: 'Too many sync wait commands'), but Tile
    emits multi-sem waits. Hoist the extras onto wait-only EventSemaphore
    carriers placed just before the instruction on the same engine —
    program order makes the split semantically identical to the fused
    multi-wait."""
    import orjson

    m = orjson.loads(bir)
    n = 0
    for f in m["functions"]:
        for blk in f["blocks"]:
            insts = blk.get("instructions")
            if not insts:
                continue
            out = []
            changed = False
            for ins in insts:
                si = ins.get("sync_info")
                ow = (si or {}).get("on_wait") or []
                if len(ow) > 1:
                    changed = True
                    for w in ow[:-1]:
                        n += 1
                        out.append(
                            {
                                "debug": ins.get("debug"),
                                "engine": ins["engine"],
                                "ins": [],
                                "outs": [],
                                "name": f"waitsplit-{n}",
                                "opcode": "EventSemaphore",
                                "sync_info": {"on_update": [], "on_wait": [w]},
                            }
                        )
                    si["on_wait"] = [ow[-1]]
                out.append(ins)
            if changed:
                blk["instructions"] = out
    return orjson.dumps(m)


_NC_CACHE = {}


def _get_nc():
    if "nc" not in _NC_CACHE:
        nc = _build()
        orig_to_json = nc.to_json_bytes
        nc.to_json_bytes = lambda: _split_multi_waits(orig_to_json())
        _NC_CACHE["nc"] = nc
    return _NC_CACHE["nc"]


def kernel(x, _trace=False, _tmpdir=None):
    assert x.shape == (32, 512, 64, 64) and x.dtype == np.float32
    x_flat = np.ascontiguousarray(x.reshape(BC_TOTAL, L))

    B64 = _blaschke_basis_f64(L)
    b_lo, b_hi = B64.min(), B64.max()
    ident = np.eye(128, dtype=np.float32)

    in_maps = []
    scales = []
    for i in range(N_CORES):
        shard = x_flat[i * BC:(i + 1) * BC]                 # [BC, L] f32
        x_lo, x_hi = float(shard.min()), float(shard.max())
        lo, hi = x_lo + b_lo, x_hi + b_hi                   # bounds on out
        s = (hi - lo) / 250.0
        c = lo - 2.5 * s                                    # q spans [2.5, 252.5]
        scales.append((s, c))

        # x' = x/s packed [NS, NJ, 128, J, SBC]
        xt = np.ascontiguousarray(shard.T) * np.float32(1.0 / s)   # [L, BC]
        xt = xt.reshape(NJ, J, 128, NS, SBC).transpose(3, 0, 2, 1, 4)
        xt = np.ascontiguousarray(xt).astype(np.float16)

        bn = np.ascontiguousarray((B64 - c) / s).astype(np.float16)  # [N, L]
        # packed bt: [128, KC*64] with bt[p, 64k+n] = s*B[n, 128k+p]
        bt = np.ascontiguousarray(
            (s * B64).T.reshape(KC, 128, N_BASIS)
            .transpose(1, 0, 2).reshape(128, KC * N_BASIS)
        ).astype(np.float16)
        in_maps.append({"xt": xt, "bt": bt, "bn": bn, "ident": ident})

    nc = _get_nc()
    res = run_bass_kernel_spmd(
        nc, in_maps, core_ids=list(range(N_CORES)), trace=_trace, tmpdir=_tmpdir
    )

    outs = []
    for i in range(N_CORES):
        s, c = scales[i]
        q = np.asarray(res.results[i]["out"])       # [NS, NJ, 128, J, SBC] u8
        # l = 128*(nj*J + j) + p; col = s_idx*SBC + c_idx
        qt = q.transpose(1, 3, 2, 0, 4).reshape(L, BC)      # [L, BC]
        out = qt.T.astype(np.float32) * np.float32(s) + np.float32(c)
        outs.append(out)                                    # [BC, L]
    out = np.concatenate(outs, axis=0).reshape(32, 512, 64, 64)
    if _trace:
        return out, res
    return out
